# revision 10
# baseline (speedup 1.0000x reference)
"""Trainium2 Bass kernel: EquivariantLayerNorm (irreps 128x0e + 64x1o + 32x2e).

Math (per row x of 480 features; scalar channels = first 128):
    m    = mean(x[:128]);  x'[:128] = x[:128] - m;  x'[128:] = x[128:]
    ss   = sum(x'^2) = sum(x^2) - 128*m^2           (groups partition the row)
    r    = 1/sqrt(ss/224)
    y    = x' * r * w_full;  y[:128] += bias
The Invariant eps terms (eps=1e-6) contribute <1e-6 relative error and are
dropped (below fp32 rounding of the reference itself).

Sharding: pure data-parallel over the row dimension across 8 NeuronCores.
Each core gets 12500 rows, padded to 12544 = 98 blocks of 128 rows
(pad rows filled with 1.0 so all stats stay finite).

Per-core kernel layout: rows on partitions (128/block), features on the free
dim. Big tiles of G=14 blocks (1792 rows, 3.4MB) per DMA. Per block:
  ACT: accum(-x_A/128) -> -mean ; accum(square(x)) -> ss
  DVE (batched over G): -128*nm^2 ; (ss+t)/224 ; ACT sqrt ; DVE reciprocal
  DVE: x_A += nm (in place) ; x = (x * r) * w_bcast (fused STT, in place)
  DVE: x[:, :, :128] += bias (one op per big tile, stride-0 broadcast)
"""

import numpy as np

DIM = 480
NS = 128          # scalar (0e) channels
NF = 224          # irrep instances
BLK = 128         # rows per block (partition dim)
N_CORES = 8

N_TOTAL = 100000
ROWS_PER_CORE = N_TOTAL // N_CORES    # 12500
G = 14                                # blocks per big tile
NBLOCKS = 98                          # ceil(12500/128)
ROWS_PAD = NBLOCKS * BLK              # 12544
NTILES = NBLOCKS // G                 # 7


def _expand_w(affine_weight):
    return np.concatenate([
        affine_weight[0:128],
        np.repeat(affine_weight[128:192], 3),
        np.repeat(affine_weight[192:224], 5),
    ]).astype(np.float32)


def _split_excess_waits(nc, dummy_sem):
    """walrus' TRN2 codegen allows at most ONE sync-wait command per engine
    instruction (S3D3_*_STRUCT).  Tile's wait assignment can emit 2+ — move
    the excess onto standalone InstEventSemaphore no-ops (same engine, placed
    immediately before), which is the same mechanism Tile's own barriers use.
    Each carries a dead increment of ``dummy_sem`` (CoreSim requires updates).
    """
    from concourse import mybir

    n = 0
    for fn in nc.m.functions:
        for blk in fn.blocks:
            out = []
            changed = False
            for inst in blk.instructions:
                si = inst.sync_info
                if si is not None and si.on_wait and len(si.on_wait) > 1:
                    waits = list(si.on_wait)
                    for w in waits[:-1]:
                        n += 1
                        ev = mybir.InstEventSemaphore(
                            name=f"I-evsplit-{n}", ins=[], outs=[])
                        ev.engine = inst.engine
                        ev.sync_info = mybir.SyncInfo(
                            on_wait=[w],
                            on_update=[mybir.SyncUpdate(
                                sync_type="semaphore", id=dummy_sem.num,
                                ant_name=dummy_sem.name,
                                update_mode="sem-inc", update_value=1,
                                update_reg=None)])
                        out.append(ev)
                    inst.sync_info = mybir.SyncInfo(
                        on_wait=[waits[-1]], on_update=list(si.on_update or []))
                    changed = True
                out.append(inst)
            if changed:
                blk.instructions = out
    return n


def build_nc(rows_pad, g, data_bufs=3):
    import concourse.bacc as bacc
    import concourse.tile as tile
    from concourse import mybir
    # All our DMAs issue from the SP HWDGE ring, which completes FIFO; using a
    # single completion-semaphore lane keeps every consumer at <=2 distinct
    # sync waits (the ISA TensorScalar struct rejects 3+) at no cost.
    from concourse import tile_sem_assignment as _tsa
    _tsa.NUM_HWDGE_SEMS = 1

    f32 = mybir.dt.float32
    Alu = mybir.AluOpType
    Act = mybir.ActivationFunctionType

    nblocks = rows_pad // BLK
    assert rows_pad % BLK == 0 and nblocks % g == 0
    ntiles = nblocks // g

    nc = bacc.Bacc("TRN2", target_bir_lowering=False, debug=False)
    evsem = nc.alloc_semaphore("evsplit_dummy")
    x = nc.dram_tensor("x", [rows_pad, DIM], f32, kind="ExternalInput").ap()
    w = nc.dram_tensor("w", [1, DIM], f32, kind="ExternalInput").ap()
    b = nc.dram_tensor("b", [1, NS], f32, kind="ExternalInput").ap()
    y = nc.dram_tensor("y", [rows_pad, DIM], f32, kind="ExternalOutput").ap()

    with tile.TileContext(nc) as tc:
        with (
            tc.tile_pool(name="const", bufs=1) as cpool,
            tc.tile_pool(name="data", bufs=data_bufs) as dpool,
            tc.tile_pool(name="stats", bufs=ntiles) as spool,
            tc.tile_pool(name="scratch", bufs=1) as zpool,
        ):
            w_t = cpool.tile([BLK, DIM], f32, name="w_t")
            nc.sync.dma_start(out=w_t[:], in_=w.broadcast_to([BLK, DIM]))
            b_t = cpool.tile([BLK, NS], f32, name="b_t")
            nc.sync.dma_start(out=b_t[:], in_=b.broadcast_to([BLK, NS]))

            da = zpool.tile([BLK, NS], f32, name="da")     # dead store (ACT accum)
            df = zpool.tile([BLK, DIM], f32, name="df")    # dead store (ACT accum)

            for t in range(ntiles):
                r0 = t * g * BLK
                src = x[r0:r0 + g * BLK, :].rearrange("(g p) d -> p g d", p=BLK)
                xt = dpool.tile([BLK, g, DIM], f32, tag="xt", name=f"xt{t}")
                nc.sync.dma_start(out=xt[:], in_=src)

                # Tiny DVE read of the freshly loaded tile: absorbs the DMAHW
                # wait on DVE so the hot TensorScalar ops below stay at <=1
                # sync wait (the ISA TS struct rejects 2+).
                dv = spool.tile([BLK, 2], f32, tag="dv", name=f"dv{t}")
                nc.vector.tensor_copy(dv[:], xt[:, g - 1, DIM - 2:DIM])

                nm = spool.tile([BLK, g], f32, tag="nm", name=f"nm{t}")
                ss = spool.tile([BLK, g], f32, tag="ss", name=f"ss{t}")
                for j in range(g):
                    nc.scalar.activation(
                        out=da[:], in_=xt[:, j, 0:NS], func=Act.Copy,
                        scale=-1.0 / NS, accum_out=nm[:, j:j + 1])
                    # (x/sqrt(224))^2 accumulated -> sum(x^2)/224 directly
                    nc.scalar.activation(
                        out=df[:], in_=xt[:, j, :], func=Act.Square,
                        scale=1.0 / float(NF) ** 0.5,
                        accum_out=ss[:, j:j + 1])

                tt = spool.tile([BLK, g], f32, tag="tt", name=f"tt{t}")
                nc.vector.scalar_tensor_tensor(
                    out=tt[:], in0=nm[:], scalar=-float(NS) / float(NF),
                    in1=nm[:], op0=Alu.mult, op1=Alu.mult)
                vv = spool.tile([BLK, g], f32, tag="vv", name=f"vv{t}")
                nc.vector.tensor_tensor(
                    out=vv[:], in0=ss[:], in1=tt[:], op=Alu.add)
                sq = spool.tile([BLK, g], f32, tag="sq", name=f"sq{t}")
                nc.scalar.activation(out=sq[:], in_=vv[:], func=Act.Sqrt)
                rr = spool.tile([BLK, g], f32, tag="rr", name=f"rr{t}")
                nc.vector.reciprocal(out=rr[:], in_=sq[:])

                for j in range(g):
                    blk = xt[:, j, :]
                    nc.vector.tensor_scalar(
                        out=xt[:, j, 0:NS], in0=xt[:, j, 0:NS],
                        scalar1=nm[:, j:j + 1], scalar2=None, op0=Alu.add)
                    nc.vector.scalar_tensor_tensor(
                        out=blk, in0=blk, scalar=rr[:, j:j + 1], in1=w_t[:],
                        op0=Alu.mult, op1=Alu.mult)

                bias_bc = b_t[:].unsqueeze(1).broadcast_to([BLK, g, NS])
                nc.vector.tensor_tensor(
                    out=xt[:, :, 0:NS], in0=xt[:, :, 0:NS], in1=bias_bc,
                    op=Alu.add)

                dst = y[r0:r0 + g * BLK, :].rearrange("(g p) d -> p g d", p=BLK)
                nc.sync.dma_start(out=dst, in_=xt[:])

    nc.compile()
    _split_excess_waits(nc, evsem)
    return nc


_NC_CACHE = {}


def _get_nc(rows_pad, g):
    key = (rows_pad, g)
    if key not in _NC_CACHE:
        _NC_CACHE[key] = build_nc(rows_pad, g)
    return _NC_CACHE[key]


def kernel(node_input, affine_weight, affine_bias):
    from concourse.bass_utils import run_bass_kernel_spmd

    node_input = np.ascontiguousarray(node_input, dtype=np.float32)
    assert node_input.shape == (N_TOTAL, DIM)
    w_full = _expand_w(np.asarray(affine_weight, dtype=np.float32)).reshape(1, DIM)
    bias = np.ascontiguousarray(
        np.asarray(affine_bias, dtype=np.float32).reshape(1, NS))

    in_maps = []
    for c in range(N_CORES):
        shard = np.ones((ROWS_PAD, DIM), dtype=np.float32)
        shard[:ROWS_PER_CORE] = node_input[c * ROWS_PER_CORE:(c + 1) * ROWS_PER_CORE]
        in_maps.append({"x": shard, "w": w_full, "b": bias})

    nc = _get_nc(ROWS_PAD, G)
    res = run_bass_kernel_spmd(nc, in_maps, core_ids=list(range(N_CORES)))
    out = np.concatenate(
        [np.asarray(res.results[c]["y"])[:ROWS_PER_CORE] for c in range(N_CORES)],
        axis=0)
    return out.astype(np.float32, copy=False)
